# revision 1
# baseline (speedup 1.0000x reference)
"""Bayesian-LSTM (blitz-style) Trainium2 Bass kernel.

Strategy:
  - Data-parallel over batch: 256 rows -> 8 cores x 32 rows; tiny weights
    replicated; the sequential scan stays local per core (sharding hint).
  - Host: sample weights (mu + softplus(rho)*eps), scatter gate columns
    from reference order [i,f,g,o] into 32-aligned partition blocks
    f@0 i@32 o@64 g@96 (engine partition bases must be 32-multiples),
    pre-transpose x per core to (D, T*BL) with (t,b) flattened b-minor so
    every DMA is fully contiguous.
  - Device per core, chunked 16 steps per PSUM bank:
      xg window = w_ih^T @ x written by PE straight into a psum bank;
      scan step: z_t = xg_t + h~ @ (w_hh/2)   [PE accumulates in place]
            gates = tanh(z*svec + bvec)  [one ACT op; sigmoid(s) is
            tanh(s/2) via svec=0.5 & halved bias for f,i,o; 1.0 for g]
            g shifted to base 32 (DVE copy) to pair with i legally
            u = (ft+1)*c~ ; v = (it+1)*g ; c~ = 0.5*u + v     [DVE STS x3]
            tc = tanh(0.5*c~)  [ACT, emitted at base 64 to pair with o]
            h~ = (ot+1)*tc                                     [DVE STS]
        with state c~ = 2c, h~ = 2h (w_hh, w_lin pre-halved on host).
      out = h~ @ (w_lin/2) + b_lin  (PE per 512-col tile + DVE add-bias)
  - Everything uses only the Tanh activation table (Copy/Identity live in
    the same set) => zero activation-table reloads in the scan.
  - Output written in (t,b) order to DRAM contiguously; host transposes.
"""

import numpy as np
from contextlib import ExitStack

B, T, D, H = 256, 2048, 32, 20
G4 = 4 * H
GP = 128  # padded gate dim: f@[0:20], i@[32:52], o@[64:84], g@[96:116]
          # (engine partition offsets must be multiples of 32)
N_CORES = 8
BL = B // N_CORES          # 32 batch rows per core
CH = 256                   # timesteps per chunk
CW = CH * BL               # 4096 free columns per chunk
MM_N = 512                 # matmul moving-free tile
NMM = CW // MM_N           # 8

_MODULE_CACHE = {}


def _build_module(t_steps=T):
    import concourse.tile as tile
    from concourse import bacc, mybir

    f32 = mybir.dt.float32
    Alu = mybir.AluOpType
    Act = mybir.ActivationFunctionType

    nch = t_steps // CH
    nc = bacc.Bacc("TRN2", target_bir_lowering=False, debug=False,
                   num_devices=N_CORES)
    xin = nc.dram_tensor("xin", [D, t_steps * BL], f32, kind="ExternalInput").ap()
    wih = nc.dram_tensor("wih", [D, GP], f32, kind="ExternalInput").ap()
    whh = nc.dram_tensor("whh", [H, GP], f32, kind="ExternalInput").ap()
    svec = nc.dram_tensor("svec", [GP, 1], f32, kind="ExternalInput").ap()
    bvec = nc.dram_tensor("bvec", [GP, 1], f32, kind="ExternalInput").ap()
    wlin = nc.dram_tensor("wlin", [H, 1], f32, kind="ExternalInput").ap()
    blin = nc.dram_tensor("blin", [1, 1], f32, kind="ExternalInput").ap()
    outd = nc.dram_tensor("out", [1, t_steps * BL], f32, kind="ExternalOutput").ap()

    WSTEPS = MM_N // BL  # 16 timesteps per psum bank window

    with tile.TileContext(nc) as tc, ExitStack() as ctx:
        misc = ctx.enter_context(tc.tile_pool(name="misc", bufs=1))
        xin_pool = ctx.enter_context(tc.tile_pool(name="xinp", bufs=2))
        hseq_pool = ctx.enter_context(tc.tile_pool(name="hseqp", bufs=2))
        gates_pool = ctx.enter_context(tc.tile_pool(name="gatesp", bufs=3))
        tmp_pool = ctx.enter_context(tc.tile_pool(name="tmpp", bufs=3))
        outsb_pool = ctx.enter_context(tc.tile_pool(name="outsbp", bufs=2))
        ps_scan = ctx.enter_context(tc.tile_pool(name="pss", bufs=4, space="PSUM"))
        ps_out = ctx.enter_context(tc.tile_pool(name="pso", bufs=2, space="PSUM"))

        wih_sb = misc.tile([D, GP], f32)
        nc.sync.dma_start(wih_sb[:], wih[:])
        # issued right after wih: the first xg fill needs only these two
        xin_w0 = misc.tile([D, MM_N], f32)
        nc.sync.dma_start(xin_w0[:], xin[:, 0:MM_N])
        whh_sb = misc.tile([H, GP], f32)
        nc.sync.dma_start(whh_sb[:], whh[:])
        svec_sb = misc.tile([GP, 1], f32)
        nc.sync.dma_start(svec_sb[:], svec[:])
        bvec_sb = misc.tile([GP, 1], f32)
        nc.sync.dma_start(bvec_sb[:], bvec[:])
        wlin_sb = misc.tile([H, 1], f32)
        nc.sync.dma_start(wlin_sb[:], wlin[:])
        blin_sb = misc.tile([1, 1], f32)
        nc.sync.dma_start(blin_sb[:], blin[:])
        cst = misc.tile([H, BL], f32)
        nc.vector.memset(cst[:], 0.0)
        h0 = misc.tile([H, BL], f32)
        nc.vector.memset(h0[:], 0.0)

        hprev = h0[:]
        for ch in range(nch):
            xin_t = xin_pool.tile([D, CW], f32)
            nc.sync.dma_start(xin_t[:], xin[:, ch * CW:(ch + 1) * CW])

            hseq = hseq_pool.tile([H, CW], f32)
            for j in range(NMM):
                # xg for the next 16 steps straight into a psum bank; the
                # recurrent matmuls then accumulate h~ @ whh in place.
                pxg = ps_scan.tile([GP, MM_N], f32)
                xsrc = xin_w0[:] if (ch == 0 and j == 0) else \
                    xin_t[:, j * MM_N:(j + 1) * MM_N]
                nc.tensor.matmul(pxg[:], wih_sb[:], xsrc,
                                 start=True, stop=True)
                for tw in range(WSTEPS):
                    t = j * WSTEPS + tw
                    sl = slice(t * BL, (t + 1) * BL)
                    zp = pxg[:, tw * BL:(tw + 1) * BL]
                    nc.tensor.matmul(zp, whh_sb[:], hprev,
                                     start=False, stop=True,
                                     skip_group_check=True)
                    # gates f@0 i@32 o@64 g@96 -> SBUF in one ACT op.
                    # SB-SB operand pairs of DVE ops must share their base
                    # partition, so g is shifted down to base 32 (1 copy)
                    # and tct is emitted at base 64 to pair with o.
                    gates = gates_pool.tile([116, BL], f32)
                    nc.scalar.activation(gates[:], zp[0:116, :], Act.Tanh,
                                         bias=bvec_sb[0:116, :],
                                         scale=svec_sb[0:116, :])
                    gg = tmp_pool.tile([52, BL], f32)
                    nc.vector.tensor_copy(gg[32:52, :], gates[96:116, :])
                    u = tmp_pool.tile([H, BL], f32)
                    v = tmp_pool.tile([H, BL], f32)
                    nc.vector.scalar_tensor_tensor(u[:], gates[0:20, :], 1.0,
                                                   cst[:], Alu.add, Alu.mult)
                    nc.vector.scalar_tensor_tensor(v[:], gates[32:52, :], 1.0,
                                                   gg[32:52, :],
                                                   Alu.add, Alu.mult)
                    nc.vector.scalar_tensor_tensor(cst[:], u[:], 0.5, v[:],
                                                   Alu.mult, Alu.add)
                    tct = tmp_pool.tile([84, BL], f32)
                    nc.scalar.activation(tct[64:84, :], cst[:], Act.Tanh,
                                         bias=0.0, scale=0.5)
                    nc.vector.scalar_tensor_tensor(hseq[:, sl],
                                                   gates[64:84, :], 1.0,
                                                   tct[64:84, :],
                                                   Alu.add, Alu.mult)
                    hprev = hseq[:, sl]

            osb = outsb_pool.tile([1, CW], f32)
            for j in range(NMM):
                po = ps_out.tile([1, MM_N], f32)
                nc.tensor.matmul(po[:], wlin_sb[:],
                                 hseq[:, j * MM_N:(j + 1) * MM_N],
                                 start=True, stop=True)
                osl = slice(j * MM_N, (j + 1) * MM_N)
                nc.vector.tensor_scalar(osb[:, osl], po[:],
                                        blin_sb[:], None, Alu.add)
                # per-slice DMA so the final chunk's store is not a
                # 12us single-partition transfer serialized at the tail
                nc.sync.dma_start(outd[:, ch * CW + j * MM_N:
                                       ch * CW + (j + 1) * MM_N],
                                  osb[:, osl])

    nc.compile()
    return nc


def get_module(t_steps=T):
    if t_steps not in _MODULE_CACHE:
        _MODULE_CACHE[t_steps] = _build_module(t_steps)
    return _MODULE_CACHE[t_steps]


def host_prep(inputs, t_steps=T):
    """Returns (in_maps, None). Pure layout + O(4k-element) param sampling."""
    x = np.asarray(inputs["x"], dtype=np.float32)

    def samp(mu, rho, eps):
        mu = np.asarray(mu, np.float32)
        rho = np.asarray(rho, np.float32)
        eps = np.asarray(eps, np.float32)
        return (mu + np.log1p(np.exp(rho)) * eps).astype(np.float32)

    w_ih = samp(inputs["w_ih_mu"], inputs["w_ih_rho"], inputs["w_ih_eps"])
    w_hh = samp(inputs["w_hh_mu"], inputs["w_hh_rho"], inputs["w_hh_eps"])
    bias = samp(inputs["b_mu"], inputs["b_rho"], inputs["b_eps"])
    w_lin = np.asarray(inputs["w_lin"], np.float32)
    b_lin = np.asarray(inputs["b_lin"], np.float32)

    # reference gate column order is [i, f, g, o]; device blocks at 0/32/64/96
    blocks = [(0, slice(20, 40)),   # f
              (32, slice(0, 20)),   # i
              (64, slice(60, 80)),  # o
              (96, slice(40, 60))]  # g

    def pad_gates(w, scale):
        out = np.zeros(w.shape[:-1] + (GP,), np.float32)
        for off, sl in blocks:
            out[..., off:off + 20] = w[..., sl] * scale
        return out

    w_ih_p = pad_gates(w_ih, 1.0)
    whh_half = pad_gates(w_hh, 0.5)
    svec = np.full((GP, 1), 0.5, np.float32)
    svec[96:116] = 1.0
    bvec = np.zeros((GP, 1), np.float32)
    for off, sl in blocks:
        sc = 1.0 if off == 96 else 0.5
        bvec[off:off + 20, 0] = bias[sl] * sc
    wlin_half = np.ascontiguousarray(w_lin * 0.5)
    blin = b_lin.reshape(1, 1).astype(np.float32)

    shared = {"wih": w_ih_p, "whh": whh_half, "svec": svec, "bvec": bvec,
              "wlin": wlin_half, "blin": blin}
    in_maps = []
    for k in range(N_CORES):
        xk = x[k * BL:(k + 1) * BL, :t_steps, :]          # (BL, t, D)
        xk = np.ascontiguousarray(xk.transpose(2, 1, 0))  # (D, t, BL)
        in_maps.append({"xin": xk.reshape(D, t_steps * BL), **shared})
    return in_maps


def assemble(results, t_steps=T):
    out = np.empty((B, t_steps, 1), np.float32)
    for k in range(N_CORES):
        r = np.asarray(results[k]["out"]).reshape(t_steps, BL)
        out[k * BL:(k + 1) * BL, :, 0] = r.T
    return out


def kernel(**inputs):
    from concourse.bass_utils import run_bass_kernel_spmd
    nc = get_module(T)
    in_maps = host_prep(inputs, T)
    try:
        res = run_bass_kernel_spmd(nc, in_maps, list(range(N_CORES)))
    except Exception:
        # transient NRT/device hiccups have been observed; retry once
        import time
        time.sleep(15)
        res = run_bass_kernel_spmd(nc, in_maps, list(range(N_CORES)))
    return assemble(res.results, T)



# revision 5
# speedup vs baseline: 4.3528x; 4.3528x over previous
"""Bayesian-LSTM (blitz-style) Trainium2 Bass kernel — time-sharded.

Strategy v2 (vs. the data-parallel v1 at 3.96ms):
  - The scan is latency-bound: ~8 chained ops x ~250ns/instr per timestep.
    Data-parallel sharding leaves T=2048 serial steps per core.
  - Time-sharding: core p computes timesteps [p*256-W, (p+1)*256) for the
    FULL batch (256 cols per op), starting from zero state W=64 steps
    early.  LSTM state influence decays ~ prod(sigmoid(f)) ~ 0.5/step, so
    the burn-in truncation error is ~1e-9 -- far below the 2e-2 gate.
    Core 0 starts exactly at t=0 from the true zero state (host slices
    its output window [0:256]; cores>0 use [W:W+256]).
    Sequential steps per core: 320 instead of 2048.
  - Engine-cost: exec time scales with the FREE dim only, so the batch
    (256) lives in the free dim and gate blocks stack in partitions.
    The 256 batch cols split into NS=2 interleaved streams of 128 cols
    whose independent dependency chains hide each other's latency.
  - Matmuls in bf16 (fp32 costs 4 cycles/row, bf16 1): w_ih, w_hh, w_lin,
    x and h are bf16; gates/state stay fp32.
  - Gate blocks f@0 i@32 o@64 g@96 (32-aligned bases); sigmoid(s) =
    (tanh(s/2)+1)/2 via ACT scale 0.5 + halved bias, states kept doubled
    (c~=2c, h~=2h, w_hh and w_lin pre-halved) => only the Tanh table.
  - Per stream-step: PE accumulates w_hh@h into the xg psum window; one
    ACT makes all gates; g shifts to base 32 on the Pool engine (gpsimd),
    freeing the Vector engine; DVE does u/v/c/h; a second ACT makes
    tanh(c).  Output projection h@w_lin on PE per 2 steps, bias-add on
    Pool, DMA out per 8 steps.
"""

import numpy as np
from contextlib import ExitStack

B, T, D, H = 256, 2048, 32, 20
GP = 128                     # padded gate dim: f@0:20 i@32:52 o@64:84 g@96:116
N_CORES = 8
W = 64                       # burn-in steps
NT = T // N_CORES            # 256 output steps per core
NSTEPS = NT + W              # 320 computed steps per core
NS = 2                       # interleaved batch streams per core
C = B // NS                  # 128 cols per stream-step
PW = 4                       # steps per psum window (PW*C = 512 f32 = 1 bank)
XW = 8                       # steps per x DMA window
OW = 8                       # steps per output DMA window

_MODULE_CACHE = {}


def _build_module(t_steps=T):
    import concourse.tile as tile
    from concourse import bacc, mybir

    f32 = mybir.dt.float32
    bf16 = mybir.dt.bfloat16
    Alu = mybir.AluOpType
    Act = mybir.ActivationFunctionType

    n = NSTEPS
    nc = bacc.Bacc("TRN2", target_bir_lowering=False, debug=False,
                   num_devices=N_CORES)
    xin = nc.dram_tensor("xin", [D, n * B], bf16, kind="ExternalInput").ap()
    wih = nc.dram_tensor("wih", [D, GP], bf16, kind="ExternalInput").ap()
    whh = nc.dram_tensor("whh", [H, GP], bf16, kind="ExternalInput").ap()
    svec = nc.dram_tensor("svec", [GP, 1], f32, kind="ExternalInput").ap()
    bvec = nc.dram_tensor("bvec", [GP, 1], f32, kind="ExternalInput").ap()
    wlin = nc.dram_tensor("wlin", [H, 1], bf16, kind="ExternalInput").ap()
    blin = nc.dram_tensor("blin", [1, 1], f32, kind="ExternalInput").ap()
    outd = nc.dram_tensor("out", [1, n * B], f32, kind="ExternalOutput").ap()

    with tile.TileContext(nc) as tc, ExitStack() as ctx:
        misc = ctx.enter_context(tc.tile_pool(name="misc", bufs=1))
        x_pool = ctx.enter_context(tc.tile_pool(name="xp", bufs=2))
        hseq_pool = ctx.enter_context(tc.tile_pool(name="hseqp", bufs=2))
        gates_pool = ctx.enter_context(tc.tile_pool(name="gatesp", bufs=4))
        gg_pool = ctx.enter_context(tc.tile_pool(name="ggp", bufs=4))
        tmp_pool = ctx.enter_context(tc.tile_pool(name="tmpp", bufs=4))
        tct_pool = ctx.enter_context(tc.tile_pool(name="tctp", bufs=4))
        osb_pool = ctx.enter_context(tc.tile_pool(name="osbp", bufs=2))
        ps_pools = [ctx.enter_context(tc.tile_pool(name=f"pss{s}", bufs=2,
                                                   space="PSUM"))
                    for s in range(NS)]
        ps_out = ctx.enter_context(tc.tile_pool(name="pso", bufs=2,
                                                space="PSUM"))

        wih_sb = misc.tile([D, GP], bf16)
        nc.sync.dma_start(wih_sb[:], wih[:])
        whh_sb = misc.tile([H, GP], bf16)
        nc.sync.dma_start(whh_sb[:], whh[:])
        svec_sb = misc.tile([GP, 1], f32)
        nc.sync.dma_start(svec_sb[:], svec[:])
        bvec_sb = misc.tile([GP, 1], f32)
        nc.sync.dma_start(bvec_sb[:], bvec[:])
        wlin_sb = misc.tile([H, 1], bf16)
        nc.sync.dma_start(wlin_sb[:], wlin[:])
        blin_sb = misc.tile([1, 1], f32)
        nc.sync.dma_start(blin_sb[:], blin[:])

        # persistent per-stream state
        cst = []
        h0 = []
        for s in range(NS):
            c_t = misc.tile([H, C], f32, name=f"cst{s}")
            nc.vector.memset(c_t[:], 0.0)
            cst.append(c_t)
            h_t = misc.tile([H, C], bf16, name=f"h0{s}")
            nc.vector.memset(h_t[:], 0.0)
            h0.append(h_t)

        nwin_x = n // XW
        x_tiles = {}

        def load_x(w):
            xt = x_pool.tile([D, XW * B], bf16, name=f"xt{w % 2}",
                             uniquify=True)
            nc.sync.dma_start(xt[:], xin[:, w * XW * B:(w + 1) * XW * B])
            x_tiles[w] = xt

        load_x(0)
        if nwin_x > 1:
            load_x(1)

        hprev = [h0[s][:] for s in range(NS)]
        pxg = [None] * NS
        hseq = None
        osb = None

        for t in range(n):
            wx = t // XW
            if t % XW == 0 and wx + 2 < nwin_x:
                load_x(wx + 2)

            tw = t % PW
            if tw == 0:
                hseq = hseq_pool.tile([H, PW * B], bf16, name='hseq')

            for s in range(NS):
                if tw == 0:
                    # xg fill for the next PW steps of this stream: rhs is
                    # the (PW, C) strided block of x cols for stream s
                    pxg[s] = ps_pools[s].tile([GP, PW * C], f32, name=f'pxg{s}')
                    xt = x_tiles[wx]
                    k0 = t % XW
                    xap = xt[:].rearrange("p (w c) -> p w c", c=B)[
                        :, k0:k0 + PW, s * C:s * C + C]
                    nc.tensor.matmul(pxg[s][:], wih_sb[:], xap,
                                     start=True, stop=True)

                zp = pxg[s][:, tw * C:(tw + 1) * C]
                nc.tensor.matmul(zp, whh_sb[:], hprev[s],
                                 start=False, stop=True,
                                 skip_group_check=True)
                gates = gates_pool.tile([116, C], f32, name='gates')
                nc.scalar.activation(gates[:], zp[0:116, :], Act.Tanh,
                                     bias=bvec_sb[0:116, :],
                                     scale=svec_sb[0:116, :])
                gg = gg_pool.tile([52, C], f32, name='gg')
                nc.gpsimd.tensor_copy(gg[32:52, :], gates[96:116, :])
                u = tmp_pool.tile([H, C], f32, name='u')
                nc.vector.scalar_tensor_tensor(u[:], gates[0:20, :], 1.0,
                                               cst[s][:], Alu.add, Alu.mult)
                v = tmp_pool.tile([H, C], f32, name='v')
                nc.vector.scalar_tensor_tensor(v[:], gates[32:52, :], 1.0,
                                               gg[32:52, :],
                                               Alu.add, Alu.mult)
                nc.vector.scalar_tensor_tensor(cst[s][:], u[:], 0.5, v[:],
                                               Alu.mult, Alu.add)
                tct = tct_pool.tile([84, C], f32, name='tct')
                nc.scalar.activation(tct[64:84, :], cst[s][:], Act.Tanh,
                                     bias=0.0, scale=0.5)
                hsl = hseq[:, tw * B + s * C:tw * B + s * C + C]
                nc.vector.scalar_tensor_tensor(hsl, gates[64:84, :], 1.0,
                                               tct[64:84, :],
                                               Alu.add, Alu.mult)
                hprev[s] = hsl

            # output projection per 2 steps (512 h cols)
            if t % OW == 0:
                osb = osb_pool.tile([1, OW * B], f32, name='osb')
            if t % 2 == 1:
                po = ps_out.tile([1, 2 * B], f32, name='po')
                nc.tensor.matmul(po[:], wlin_sb[:],
                                 hseq[:, (tw - 1) * B:(tw + 1) * B],
                                 start=True, stop=True)
                osl = slice((t % OW - 1) * B, (t % OW + 1) * B)
                nc.gpsimd.tensor_scalar(osb[:, osl], po[:],
                                        blin_sb[:], None, Alu.add)
            if t % OW == OW - 1:
                w0 = (t - (OW - 1)) * B
                nc.sync.dma_start(outd[:, w0:w0 + OW * B], osb[:])

    nc.compile()
    return nc


def get_module(t_steps=T):
    if t_steps not in _MODULE_CACHE:
        _MODULE_CACHE[t_steps] = _build_module(t_steps)
    return _MODULE_CACHE[t_steps]


def host_prep(inputs, t_steps=T):
    import ml_dtypes
    bf16 = ml_dtypes.bfloat16
    x = np.asarray(inputs["x"], dtype=np.float32)

    def samp(mu, rho, eps):
        mu = np.asarray(mu, np.float32)
        rho = np.asarray(rho, np.float32)
        eps = np.asarray(eps, np.float32)
        return (mu + np.log1p(np.exp(rho)) * eps).astype(np.float32)

    w_ih = samp(inputs["w_ih_mu"], inputs["w_ih_rho"], inputs["w_ih_eps"])
    w_hh = samp(inputs["w_hh_mu"], inputs["w_hh_rho"], inputs["w_hh_eps"])
    bias = samp(inputs["b_mu"], inputs["b_rho"], inputs["b_eps"])
    w_lin = np.asarray(inputs["w_lin"], np.float32)
    b_lin = np.asarray(inputs["b_lin"], np.float32)

    # reference gate column order is [i, f, g, o]; device blocks at 0/32/64/96
    blocks = [(0, slice(20, 40)),   # f
              (32, slice(0, 20)),   # i
              (64, slice(60, 80)),  # o
              (96, slice(40, 60))]  # g

    def pad_gates(w, scale):
        out = np.zeros(w.shape[:-1] + (GP,), np.float32)
        for off, sl in blocks:
            out[..., off:off + 20] = w[..., sl] * scale
        return out

    w_ih_p = pad_gates(w_ih, 1.0).astype(bf16)
    whh_half = pad_gates(w_hh, 0.5).astype(bf16)
    svec = np.full((GP, 1), 0.5, np.float32)
    svec[96:116] = 1.0
    bvec = np.zeros((GP, 1), np.float32)
    for off, sl in blocks:
        sc = 1.0 if off == 96 else 0.5
        bvec[off:off + 20, 0] = bias[sl] * sc
    wlin_half = np.ascontiguousarray(w_lin * 0.5).astype(bf16)
    blin = b_lin.reshape(1, 1).astype(np.float32)

    shared = {"wih": w_ih_p, "whh": whh_half, "svec": svec, "bvec": bvec,
              "wlin": wlin_half, "blin": blin}
    x16 = x.astype(bf16)
    in_maps = []
    for p in range(N_CORES):
        start = 0 if p == 0 else p * NT - W
        xc = x16[:, start:start + NSTEPS, :]          # (B, n, D)
        xc = np.ascontiguousarray(xc.transpose(2, 1, 0))  # (D, n, B)
        in_maps.append({"xin": xc.reshape(D, NSTEPS * B), **shared})
    return in_maps


def assemble(results, t_steps=T):
    out = np.empty((B, t_steps, 1), np.float32)
    for p in range(N_CORES):
        r = np.asarray(results[p]["out"]).reshape(NSTEPS, B)
        w0 = 0 if p == 0 else W
        out[:, p * NT:(p + 1) * NT, 0] = r[w0:w0 + NT, :].T
    return out


def kernel(**inputs):
    from concourse.bass_utils import run_bass_kernel_spmd
    nc = get_module(T)
    in_maps = host_prep(inputs, T)
    try:
        res = run_bass_kernel_spmd(nc, in_maps, list(range(N_CORES)))
    except Exception:
        # transient NRT/device hiccups have been observed; retry once
        import time
        time.sleep(15)
        res = run_bass_kernel_spmd(nc, in_maps, list(range(N_CORES)))
    return assemble(res.results, T)
